# revision 27
# baseline (speedup 1.0000x reference)
"""Bahdanau-style content-based attention on Trainium2, data-parallel over 8 cores.

Reference math (per batch b, H=128, N=2048):
    hidden = concat(static[b], dynamic[b], broadcast(decoder[b]))   # (3H, N)
    Wh     = tanh(W @ hidden)                                       # (H, N)
    scores = v . Wh                                                 # (N,)
    out[b] = softmax(scores)                                        # (1, N)

Algebraic simplifications baked into the kernel:
  * The decoder third of W @ hidden has identical columns, so it collapses to a
    per-batch bias vector W_c @ decoder[b], fused into tanh via the scalar
    engine's per-partition bias. Removes 1/3 of matmul work.
  * The v-reduction for batch b uses a one-hot stationary operand (v placed in
    column b, zeros elsewhere) so the resulting scores row lands directly in
    PSUM partition b of a shared (32, N) accumulator — all 32 batches
    accumulate into one tile and softmax runs once at full lane occupancy.

Sharding: batch dim B=256 split across 8 cores (32 each); v/W replicated.
"""

from contextlib import ExitStack

import numpy as np

import concourse.bass as bass
import concourse.tile as tile
from concourse import bacc, mybir
from concourse.bass_utils import run_bass_kernel_spmd

B, H, N = 256, 128, 2048
N_CORES = 8
BPC = B // N_CORES  # 32 batches per core
NCH = 512           # matmul free-dim chunk = one fp32 PSUM bank
NCHUNKS = N // NCH  # 4

F32 = mybir.dt.float32
F32R = mybir.dt.float32r
AF = mybir.ActivationFunctionType
AX = mybir.AxisListType


def build_bass(inpool_bufs: int = 6, whpool_bufs: int = 3,
               psum_wh_bufs: int = 2, flush_lag: int = 2, n_fillers: int = 8,
               finalize: bool = True) -> bass.Bass:
    # Bacc's finalize pipeline splits multi-semaphore waits (TRN2 allows one
    # wait per instruction; fp32r matmuls have no ldweights to absorb extras).
    nc = bacc.Bacc(None, target_bir_lowering=False)

    # Tensors feeding fp32r matmuls are declared float32r end-to-end (the BIR
    # verifier requires fp32r matmul operands to come from fp32r producers);
    # the host pre-rounds their payload to e8m11.
    static_d = nc.dram_tensor("static", [BPC, H, N], F32R, kind="ExternalInput")
    dynamic_d = nc.dram_tensor("dynamic", [BPC, H, N], F32R, kind="ExternalInput")
    dect_d = nc.dram_tensor("dect", [H, BPC], F32, kind="ExternalInput")
    wst_d = nc.dram_tensor("wst", [H, H], F32R, kind="ExternalInput")
    wdt_d = nc.dram_tensor("wdt", [H, H], F32R, kind="ExternalInput")
    wct_d = nc.dram_tensor("wct", [H, H], F32, kind="ExternalInput")
    vband_d = nc.dram_tensor("vband", [H, BPC, BPC], F32R, kind="ExternalInput")
    out_d = nc.dram_tensor("out", [BPC, N], F32, kind="ExternalOutput")

    with tile.TileContext(nc) as tc:
        with ExitStack() as ctx:
            singles = ctx.enter_context(tc.tile_pool(name="singles", bufs=1))
            inpool = ctx.enter_context(tc.tile_pool(name="inpool", bufs=inpool_bufs))
            whpool = ctx.enter_context(tc.tile_pool(name="whpool", bufs=whpool_bufs))
            psum_wh = ctx.enter_context(
                tc.tile_pool(name="psum_wh", bufs=psum_wh_bufs, space="PSUM"))
            psum_sc = ctx.enter_context(
                tc.tile_pool(name="psum_sc", bufs=1, space="PSUM"))

            in_tiles = {}
            wst = singles.tile([H, H], F32R)
            wdt = singles.tile([H, H], F32R)
            wct = singles.tile([H, H], F32)
            dect = singles.tile([H, BPC], F32)

            def issue_input_dma(b):
                s_t = inpool.tile([H, N], F32R, tag="s")
                nc.sync.dma_start(out=s_t, in_=static_d[b])
                d_t = inpool.tile([H, N], F32R, tag="d")
                nc.sync.dma_start(out=d_t, in_=dynamic_d[b])
                in_tiles[b] = (s_t, d_t)

            # Batch 0's input is the critical path to the first real matmul;
            # dispatch it before everything else. The small weight/decoder
            # tensors ride along cheaply.
            issue_input_dma(0)
            nc.sync.dma_start(out=wst, in_=wst_d[:])
            nc.sync.dma_start(out=wdt, in_=wdt_d[:])
            nc.sync.dma_start(out=wct, in_=wct_d[:])
            nc.sync.dma_start(out=dect, in_=dect_d[:])
            issue_input_dma(1)

            vband = singles.tile([H, BPC, BPC], F32R)
            nc.sync.dma_start(out=vband, in_=vband_d[:])
            # Flat (H, 512) view of vband used as a dummy moving operand for
            # HAM warm-keeper matmuls (always resident, no DMA dependency).
            vflat = vband[:, 0:BPC // 2, :].rearrange("p a b -> p (a b)")

            # Per-batch bias vectors: biasT[:, b] = W_c @ decoder[b].
            # Full-precision fp32 matmul (tiny: 32 output columns).
            bias_ps = psum_wh.tile([H, BPC], F32, tag="ps")
            nc.tensor.matmul(bias_ps, wct[:], dect[:], start=True, stop=True)
            bias_sb = singles.tile([H, BPC], F32)
            nc.vector.tensor_copy(out=bias_sb, in_=bias_ps)

            # Scores accumulator: row b holds batch b's scores.
            sc_ps = psum_sc.tile([BPC, N], F32)

            def flush_scores(bb, whb):
                for c in range(NCHUNKS):
                    sl = bass.ts(c, NCH)
                    nc.tensor.matmul(sc_ps[:, sl], vband[:, bb, :], whb[:, sl],
                                     start=(bb == 0), stop=(bb == BPC - 1))

            pending = []
            next_dma = 2
            for b in range(BPC):
                while next_dma < min(BPC, b + inpool_bufs):
                    issue_input_dma(next_dma)
                    next_dma += 1
                s_t, d_t = in_tiles.pop(b)
                wh = whpool.tile([H, N], F32R, tag="wh")
                # Two (H, 1024) PSUM tiles per batch; tanh reads each tile as
                # one wide op (amortizes ACT per-instruction overhead).
                for half in range(2):
                    ps = psum_wh.tile([H, 2 * NCH], F32, tag="ps")
                    if half == 0:
                        # HAM warm-keepers: throwaway matmuls on resident data
                        # keep the PE array active while it waits for this
                        # batch's input DMA, so the clock gate stays at 8/8
                        # (cold PE at 1.2 GHz cannot keep up with DMA pace,
                        # warm PE at 2.4 GHz has ~2x slack). Tapered off for
                        # the final batches, whose input is already buffered
                        # by the time the DMA stream finishes.
                        nf = n_fillers if b < BPC - 8 else (
                            n_fillers // 2 if b < BPC - 5 else 0)
                        for _ in range(nf):
                            nc.tensor.matmul(ps[0:BPC, 0:NCH],
                                             vband[:, 0, :], vflat,
                                             start=True, stop=True)
                    for sub in range(2):
                        c = 2 * half + sub
                        sl = bass.ts(c, NCH)
                        psl = bass.ts(sub, NCH)
                        nc.tensor.matmul(ps[:, psl], wst[:], s_t[:, sl],
                                         start=True, stop=False)
                        nc.tensor.matmul(ps[:, psl], wdt[:], d_t[:, sl],
                                         start=False, stop=True)
                    nc.scalar.activation(out=wh[:, bass.ts(half, 2 * NCH)],
                                         in_=ps, func=AF.Tanh,
                                         bias=bias_sb[:, b:b + 1], scale=1.0)
                pending.append((b, wh))
                # Keep the v-reduction a couple batches behind so the PE never
                # waits on the ACT engine's tanh of the current batch.
                if len(pending) > flush_lag:
                    flush_scores(*pending.pop(0))
            while pending:
                flush_scores(*pending.pop(0))

            # Row softmax over the 32 per-core batches, reading scores from
            # PSUM. No max subtraction: |scores| <= sum|v_h| < ~7, so exp is
            # comfortably within fp32 range and softmax is shift-invariant.
            exp_sb = singles.tile([BPC, N], F32)
            ssum = singles.tile([BPC, 1], F32)
            nc.scalar.activation(out=exp_sb, in_=sc_ps, func=AF.Exp,
                                 bias=0.0, scale=1.0, accum_out=ssum[:])
            rsum = singles.tile([BPC, 1], F32)
            nc.vector.reciprocal(out=rsum, in_=ssum)
            out_sb = singles.tile([BPC, N], F32)
            nc.vector.tensor_scalar_mul(out=out_sb, in0=exp_sb, scalar1=rsum[:])
            nc.sync.dma_start(out=out_d[:], in_=out_sb)

    if finalize:
        nc.finalize()
    return nc


def round_fp32r(x):
    """Round fp32 to fp32r (e8m11): round-to-nearest-even, drop low 12 bits."""
    u = np.ascontiguousarray(x, np.float32).view(np.uint32)
    drop = 12
    half = np.uint32(1 << (drop - 1))
    lsb = (u >> drop) & np.uint32(1)
    u2 = (u + (half - np.uint32(1)) + lsb) & np.uint32(~((1 << drop) - 1) & 0xFFFFFFFF)
    return u2.view(np.float32)


def make_in_maps(static_hidden, dynamic_hidden, decoder_hidden, v, W):
    static_hidden = round_fp32r(np.asarray(static_hidden, np.float32))
    dynamic_hidden = round_fp32r(np.asarray(dynamic_hidden, np.float32))
    decoder_hidden = np.asarray(decoder_hidden, np.float32)
    v = np.asarray(v, np.float32)
    W = np.asarray(W, np.float32)

    w0 = W[0]  # (H, 3H)
    wst = round_fp32r(np.ascontiguousarray(w0[:, 0:H].T))   # lhsT, static part
    wdt = round_fp32r(np.ascontiguousarray(w0[:, H:2 * H].T))
    wct = np.ascontiguousarray(w0[:, 2 * H:3 * H].T)
    dect = np.ascontiguousarray(decoder_hidden.T)     # (H, B)

    # vband[:, b, :] is the stationary operand for batch b's v-reduction:
    # v in column b, zeros elsewhere, so lhsT.T @ Wh lands in PSUM row b.
    vband = np.zeros((H, BPC, BPC), np.float32)
    for bb in range(BPC):
        vband[:, bb, bb] = v[0, 0]
    vband = round_fp32r(vband)

    in_maps = []
    for c in range(N_CORES):
        sl = slice(c * BPC, (c + 1) * BPC)
        in_maps.append({
            "static": static_hidden[sl],
            "dynamic": dynamic_hidden[sl],
            "dect": np.ascontiguousarray(dect[:, sl]),
            "wst": wst,
            "wdt": wdt,
            "wct": wct,
            "vband": vband,
        })
    return in_maps


def assemble_output(results):
    outs = [np.asarray(results[c]["out"]) for c in range(N_CORES)]
    return np.concatenate(outs, axis=0)[:, None, :].astype(np.float32)


def kernel(static_hidden, dynamic_hidden, decoder_hidden, v, W):
    nc = build_bass()
    in_maps = make_in_maps(static_hidden, dynamic_hidden, decoder_hidden, v, W)
    res = run_bass_kernel_spmd(nc, in_maps, core_ids=list(range(N_CORES)))
    return assemble_output(res.results)


# revision 28
# speedup vs baseline: 1.0872x; 1.0872x over previous
"""Bahdanau-style content-based attention on Trainium2, data-parallel over 8 cores.

Reference math (per batch b, H=128, N=2048):
    hidden = concat(static[b], dynamic[b], broadcast(decoder[b]))   # (3H, N)
    Wh     = tanh(W @ hidden)                                       # (H, N)
    scores = v . Wh                                                 # (N,)
    out[b] = softmax(scores)                                        # (1, N)

Algebraic simplifications baked into the kernel:
  * The decoder third of W @ hidden has identical columns, so it collapses to a
    per-batch bias vector W_c @ decoder[b], fused into tanh via the scalar
    engine's per-partition bias. Removes 1/3 of matmul work.
  * The v-reduction for batch b uses a one-hot stationary operand (v placed in
    column b, zeros elsewhere) so the resulting scores row lands directly in
    PSUM partition b of a shared (32, N) accumulator — all 32 batches
    accumulate into one tile and softmax runs once at full lane occupancy.

Sharding: batch dim B=256 split across 8 cores (32 each); v/W replicated.
"""

from contextlib import ExitStack

import numpy as np

import concourse.bass as bass
import concourse.tile as tile
from concourse import bacc, mybir
from concourse.bass_utils import run_bass_kernel_spmd

B, H, N = 256, 128, 2048
N_CORES = 8
BPC = B // N_CORES  # 32 batches per core
NCH = 512           # matmul free-dim chunk = one fp32 PSUM bank
NCHUNKS = N // NCH  # 4

F32 = mybir.dt.float32
F32R = mybir.dt.float32r
AF = mybir.ActivationFunctionType
AX = mybir.AxisListType


def build_bass(inpool_bufs: int = 7, whpool_bufs: int = 3,
               psum_wh_bufs: int = 2, flush_lag: int = 2, n_fillers: int = 8,
               finalize: bool = True) -> bass.Bass:
    # Bacc's finalize pipeline splits multi-semaphore waits (TRN2 allows one
    # wait per instruction; fp32r matmuls have no ldweights to absorb extras).
    nc = bacc.Bacc(None, target_bir_lowering=False)

    # Tensors feeding fp32r matmuls are declared float32r end-to-end (the BIR
    # verifier requires fp32r matmul operands to come from fp32r producers);
    # the host pre-rounds their payload to e8m11.
    static_d = nc.dram_tensor("static", [BPC, H, N], F32R, kind="ExternalInput")
    dynamic_d = nc.dram_tensor("dynamic", [BPC, H, N], F32R, kind="ExternalInput")
    dect_d = nc.dram_tensor("dect", [H, BPC], F32, kind="ExternalInput")
    wst_d = nc.dram_tensor("wst", [H, H], F32R, kind="ExternalInput")
    wdt_d = nc.dram_tensor("wdt", [H, H], F32R, kind="ExternalInput")
    wct_d = nc.dram_tensor("wct", [H, H], F32, kind="ExternalInput")
    vband_d = nc.dram_tensor("vband", [H, BPC, BPC], F32R, kind="ExternalInput")
    out_d = nc.dram_tensor("out", [BPC, N], F32, kind="ExternalOutput")

    with tile.TileContext(nc) as tc:
        with ExitStack() as ctx:
            singles = ctx.enter_context(tc.tile_pool(name="singles", bufs=1))
            inpool = ctx.enter_context(tc.tile_pool(name="inpool", bufs=inpool_bufs))
            whpool = ctx.enter_context(tc.tile_pool(name="whpool", bufs=whpool_bufs))
            psum_wh = ctx.enter_context(
                tc.tile_pool(name="psum_wh", bufs=psum_wh_bufs, space="PSUM"))
            psum_sc = ctx.enter_context(
                tc.tile_pool(name="psum_sc", bufs=1, space="PSUM"))

            in_tiles = {}
            wst = singles.tile([H, H], F32R)
            wdt = singles.tile([H, H], F32R)
            wct = singles.tile([H, H], F32)
            dect = singles.tile([H, BPC], F32)

            def issue_input_dma(b):
                s_t = inpool.tile([H, N], F32R, tag="s")
                nc.sync.dma_start(out=s_t, in_=static_d[b])
                d_t = inpool.tile([H, N], F32R, tag="d")
                nc.sync.dma_start(out=d_t, in_=dynamic_d[b])
                in_tiles[b] = (s_t, d_t)

            # Batch 0's input is the critical path to the first real matmul;
            # dispatch it before everything else. The small weight/decoder
            # tensors ride along cheaply.
            issue_input_dma(0)
            nc.sync.dma_start(out=wst, in_=wst_d[:])
            nc.sync.dma_start(out=wdt, in_=wdt_d[:])
            nc.sync.dma_start(out=wct, in_=wct_d[:])
            nc.sync.dma_start(out=dect, in_=dect_d[:])
            issue_input_dma(1)

            vband = singles.tile([H, BPC, BPC], F32R)
            nc.sync.dma_start(out=vband, in_=vband_d[:])
            # Flat (H, 512) view of vband used as a dummy moving operand for
            # HAM warm-keeper matmuls (always resident, no DMA dependency).
            vflat = vband[:, 0:BPC // 2, :].rearrange("p a b -> p (a b)")

            # Per-batch bias vectors: biasT[:, b] = W_c @ decoder[b].
            # Full-precision fp32 matmul (tiny: 32 output columns).
            bias_ps = psum_wh.tile([H, BPC], F32, tag="ps")
            nc.tensor.matmul(bias_ps, wct[:], dect[:], start=True, stop=True)
            bias_sb = singles.tile([H, BPC], F32)
            nc.vector.tensor_copy(out=bias_sb, in_=bias_ps)

            # Scores accumulator: row b holds batch b's scores.
            sc_ps = psum_sc.tile([BPC, N], F32)

            def flush_scores(bb, whb):
                for c in range(NCHUNKS):
                    sl = bass.ts(c, NCH)
                    nc.tensor.matmul(sc_ps[:, sl], vband[:, bb, :], whb[:, sl],
                                     start=(bb == 0), stop=(bb == BPC - 1))

            pending = []
            next_dma = 2
            for b in range(BPC):
                while next_dma < min(BPC, b + inpool_bufs):
                    issue_input_dma(next_dma)
                    next_dma += 1
                s_t, d_t = in_tiles.pop(b)
                wh = whpool.tile([H, N], F32R, tag="wh")
                # Two (H, 1024) PSUM tiles per batch; tanh reads each tile as
                # one wide op (amortizes ACT per-instruction overhead).
                for half in range(2):
                    ps = psum_wh.tile([H, 2 * NCH], F32, tag="ps")
                    if half == 0:
                        # HAM warm-keepers: throwaway matmuls on resident data
                        # keep the PE array active while it waits for this
                        # batch's input DMA, so the clock gate stays at 8/8
                        # (cold PE at 1.2 GHz cannot keep up with DMA pace,
                        # warm PE at 2.4 GHz has ~2x slack). Tapered off for
                        # the final batches, whose input is already buffered
                        # by the time the DMA stream finishes.
                        nf = n_fillers if b < BPC - 8 else (
                            n_fillers // 2 if b < BPC - 5 else 0)
                        for _ in range(nf):
                            nc.tensor.matmul(ps[0:BPC, 0:NCH],
                                             vband[:, 0, :], vflat,
                                             start=True, stop=True)
                    for sub in range(2):
                        c = 2 * half + sub
                        sl = bass.ts(c, NCH)
                        psl = bass.ts(sub, NCH)
                        nc.tensor.matmul(ps[:, psl], wst[:], s_t[:, sl],
                                         start=True, stop=False)
                        nc.tensor.matmul(ps[:, psl], wdt[:], d_t[:, sl],
                                         start=False, stop=True)
                    nc.scalar.activation(out=wh[:, bass.ts(half, 2 * NCH)],
                                         in_=ps, func=AF.Tanh,
                                         bias=bias_sb[:, b:b + 1], scale=1.0)
                pending.append((b, wh))
                # Keep the v-reduction a couple batches behind so the PE never
                # waits on the ACT engine's tanh of the current batch.
                if len(pending) > flush_lag:
                    flush_scores(*pending.pop(0))
            while pending:
                flush_scores(*pending.pop(0))

            # Row softmax over the 32 per-core batches, reading scores from
            # PSUM. No max subtraction: |scores| <= sum|v_h| < ~7, so exp is
            # comfortably within fp32 range and softmax is shift-invariant.
            exp_sb = singles.tile([BPC, N], F32)
            ssum = singles.tile([BPC, 1], F32)
            nc.scalar.activation(out=exp_sb, in_=sc_ps, func=AF.Exp,
                                 bias=0.0, scale=1.0, accum_out=ssum[:])
            rsum = singles.tile([BPC, 1], F32)
            nc.vector.reciprocal(out=rsum, in_=ssum)
            out_sb = singles.tile([BPC, N], F32)
            nc.vector.tensor_scalar_mul(out=out_sb, in0=exp_sb, scalar1=rsum[:])
            nc.sync.dma_start(out=out_d[:], in_=out_sb)

    if finalize:
        nc.finalize()
    return nc


def round_fp32r(x):
    """Round fp32 to fp32r (e8m11): round-to-nearest-even, drop low 12 bits."""
    u = np.ascontiguousarray(x, np.float32).view(np.uint32)
    drop = 12
    half = np.uint32(1 << (drop - 1))
    lsb = (u >> drop) & np.uint32(1)
    u2 = (u + (half - np.uint32(1)) + lsb) & np.uint32(~((1 << drop) - 1) & 0xFFFFFFFF)
    return u2.view(np.float32)


def make_in_maps(static_hidden, dynamic_hidden, decoder_hidden, v, W):
    static_hidden = round_fp32r(np.asarray(static_hidden, np.float32))
    dynamic_hidden = round_fp32r(np.asarray(dynamic_hidden, np.float32))
    decoder_hidden = np.asarray(decoder_hidden, np.float32)
    v = np.asarray(v, np.float32)
    W = np.asarray(W, np.float32)

    w0 = W[0]  # (H, 3H)
    wst = round_fp32r(np.ascontiguousarray(w0[:, 0:H].T))   # lhsT, static part
    wdt = round_fp32r(np.ascontiguousarray(w0[:, H:2 * H].T))
    wct = np.ascontiguousarray(w0[:, 2 * H:3 * H].T)
    dect = np.ascontiguousarray(decoder_hidden.T)     # (H, B)

    # vband[:, b, :] is the stationary operand for batch b's v-reduction:
    # v in column b, zeros elsewhere, so lhsT.T @ Wh lands in PSUM row b.
    vband = np.zeros((H, BPC, BPC), np.float32)
    for bb in range(BPC):
        vband[:, bb, bb] = v[0, 0]
    vband = round_fp32r(vband)

    in_maps = []
    for c in range(N_CORES):
        sl = slice(c * BPC, (c + 1) * BPC)
        in_maps.append({
            "static": static_hidden[sl],
            "dynamic": dynamic_hidden[sl],
            "dect": np.ascontiguousarray(dect[:, sl]),
            "wst": wst,
            "wdt": wdt,
            "wct": wct,
            "vband": vband,
        })
    return in_maps


def assemble_output(results):
    outs = [np.asarray(results[c]["out"]) for c in range(N_CORES)]
    return np.concatenate(outs, axis=0)[:, None, :].astype(np.float32)


def kernel(static_hidden, dynamic_hidden, decoder_hidden, v, W):
    nc = build_bass()
    in_maps = make_in_maps(static_hidden, dynamic_hidden, decoder_hidden, v, W)
    res = run_bass_kernel_spmd(nc, in_maps, core_ids=list(range(N_CORES)))
    return assemble_output(res.results)


# revision 30
# speedup vs baseline: 1.1039x; 1.0153x over previous
"""Bahdanau-style content-based attention on Trainium2, data-parallel over 8 cores.

Reference math (per batch b, H=128, N=2048):
    hidden = concat(static[b], dynamic[b], broadcast(decoder[b]))   # (3H, N)
    Wh     = tanh(W @ hidden)                                       # (H, N)
    scores = v . Wh                                                 # (N,)
    out[b] = softmax(scores)                                        # (1, N)

Algebraic simplifications baked into the kernel:
  * The decoder third of W @ hidden has identical columns, so it collapses to a
    per-batch bias vector W_c @ decoder[b], fused into tanh via the scalar
    engine's per-partition bias. Removes 1/3 of matmul work.
  * The v-reduction for batch b uses a one-hot stationary operand (v placed in
    column b, zeros elsewhere) so the resulting scores row lands directly in
    PSUM partition b of a shared (32, N) accumulator — all 32 batches
    accumulate into one tile and softmax runs once at full lane occupancy.

Sharding: batch dim B=256 split across 8 cores (32 each); v/W replicated.
"""

from contextlib import ExitStack

import numpy as np

import concourse.bass as bass
import concourse.tile as tile
from concourse import bacc, mybir
from concourse.bass_utils import run_bass_kernel_spmd

B, H, N = 256, 128, 2048
N_CORES = 8
BPC = B // N_CORES  # 32 batches per core
NCH = 512           # matmul free-dim chunk = one fp32 PSUM bank
NCHUNKS = N // NCH  # 4

F32 = mybir.dt.float32
F32R = mybir.dt.float32r
AF = mybir.ActivationFunctionType
AX = mybir.AxisListType


def build_bass(inpool_bufs: int = 7, whpool_bufs: int = 3,
               psum_wh_bufs: int = 2, flush_lag: int = 2, n_fillers: int = 7,
               finalize: bool = True) -> bass.Bass:
    # Bacc's finalize pipeline splits multi-semaphore waits (TRN2 allows one
    # wait per instruction; fp32r matmuls have no ldweights to absorb extras).
    nc = bacc.Bacc(None, target_bir_lowering=False)

    # Tensors feeding fp32r matmuls are declared float32r end-to-end (the BIR
    # verifier requires fp32r matmul operands to come from fp32r producers);
    # the host pre-rounds their payload to e8m11.
    static_d = nc.dram_tensor("static", [BPC, H, N], F32R, kind="ExternalInput")
    dynamic_d = nc.dram_tensor("dynamic", [BPC, H, N], F32R, kind="ExternalInput")
    dect_d = nc.dram_tensor("dect", [H, BPC], F32, kind="ExternalInput")
    wst_d = nc.dram_tensor("wst", [H, H], F32R, kind="ExternalInput")
    wdt_d = nc.dram_tensor("wdt", [H, H], F32R, kind="ExternalInput")
    wct_d = nc.dram_tensor("wct", [H, H], F32, kind="ExternalInput")
    vband_d = nc.dram_tensor("vband", [H, BPC, BPC], F32R, kind="ExternalInput")
    out_d = nc.dram_tensor("out", [BPC, N], F32, kind="ExternalOutput")

    with tile.TileContext(nc) as tc:
        with ExitStack() as ctx:
            singles = ctx.enter_context(tc.tile_pool(name="singles", bufs=1))
            inpool = ctx.enter_context(tc.tile_pool(name="inpool", bufs=inpool_bufs))
            whpool = ctx.enter_context(tc.tile_pool(name="whpool", bufs=whpool_bufs))
            psum_wh = ctx.enter_context(
                tc.tile_pool(name="psum_wh", bufs=psum_wh_bufs, space="PSUM"))
            psum_sc = ctx.enter_context(
                tc.tile_pool(name="psum_sc", bufs=1, space="PSUM"))

            in_tiles = {}
            wst = singles.tile([H, H], F32R)
            wdt = singles.tile([H, H], F32R)
            wct = singles.tile([H, H], F32)
            dect = singles.tile([H, BPC], F32)

            def issue_input_dma(b):
                s_t = inpool.tile([H, N], F32R, tag="s")
                nc.sync.dma_start(out=s_t, in_=static_d[b])
                d_t = inpool.tile([H, N], F32R, tag="d")
                nc.sync.dma_start(out=d_t, in_=dynamic_d[b])
                in_tiles[b] = (s_t, d_t)

            # Batch 0's input is the critical path to the first real matmul;
            # dispatch it before everything else. The small weight/decoder
            # tensors ride along cheaply.
            issue_input_dma(0)
            nc.sync.dma_start(out=wst, in_=wst_d[:])
            nc.sync.dma_start(out=wdt, in_=wdt_d[:])
            nc.sync.dma_start(out=wct, in_=wct_d[:])
            nc.sync.dma_start(out=dect, in_=dect_d[:])
            issue_input_dma(1)

            vband = singles.tile([H, BPC, BPC], F32R)
            nc.sync.dma_start(out=vband, in_=vband_d[:])
            # Flat (H, 512) view of vband used as a dummy moving operand for
            # HAM warm-keeper matmuls (always resident, no DMA dependency).
            vflat = vband[:, 0:BPC // 2, :].rearrange("p a b -> p (a b)")

            # Per-batch bias vectors: biasT[:, b] = W_c @ decoder[b].
            # Full-precision fp32 matmul (tiny: 32 output columns).
            bias_ps = psum_wh.tile([H, BPC], F32, tag="ps")
            nc.tensor.matmul(bias_ps, wct[:], dect[:], start=True, stop=True)
            bias_sb = singles.tile([H, BPC], F32)
            nc.vector.tensor_copy(out=bias_sb, in_=bias_ps)

            # Scores accumulator: row b holds batch b's scores.
            sc_ps = psum_sc.tile([BPC, N], F32)

            def flush_scores(bb, whb):
                for c in range(NCHUNKS):
                    sl = bass.ts(c, NCH)
                    nc.tensor.matmul(sc_ps[:, sl], vband[:, bb, :], whb[:, sl],
                                     start=(bb == 0), stop=(bb == BPC - 1))

            pending = []
            next_dma = 2
            for b in range(BPC):
                while next_dma < min(BPC, b + inpool_bufs):
                    issue_input_dma(next_dma)
                    next_dma += 1
                s_t, d_t = in_tiles.pop(b)
                wh = whpool.tile([H, N], F32R, tag="wh")
                # Two (H, 1024) PSUM tiles per batch; tanh reads each tile as
                # one wide op (amortizes ACT per-instruction overhead).
                for half in range(2):
                    ps = psum_wh.tile([H, 2 * NCH], F32, tag="ps")
                    if half == 0:
                        # HAM warm-keepers: throwaway matmuls on resident data
                        # keep the PE array active while it waits for this
                        # batch's input DMA, so the clock gate stays at 8/8
                        # (cold PE at 1.2 GHz cannot keep up with DMA pace,
                        # warm PE at 2.4 GHz has ~2x slack). Tapered off for
                        # the final batches, whose input is already buffered
                        # by the time the DMA stream finishes.
                        nf = n_fillers if b < BPC - 10 else (
                            n_fillers // 2 if b < BPC - 5 else 0)
                        for _ in range(nf):
                            nc.tensor.matmul(ps[0:BPC, 0:NCH],
                                             vband[:, 0, :], vflat,
                                             start=True, stop=True)
                    for sub in range(2):
                        c = 2 * half + sub
                        sl = bass.ts(c, NCH)
                        psl = bass.ts(sub, NCH)
                        nc.tensor.matmul(ps[:, psl], wst[:], s_t[:, sl],
                                         start=True, stop=False)
                        nc.tensor.matmul(ps[:, psl], wdt[:], d_t[:, sl],
                                         start=False, stop=True)
                    nc.scalar.activation(out=wh[:, bass.ts(half, 2 * NCH)],
                                         in_=ps, func=AF.Tanh,
                                         bias=bias_sb[:, b:b + 1], scale=1.0)
                pending.append((b, wh))
                # Keep the v-reduction a couple batches behind so the PE never
                # waits on the ACT engine's tanh of the current batch.
                if len(pending) > flush_lag:
                    flush_scores(*pending.pop(0))
            while pending:
                flush_scores(*pending.pop(0))

            # Row softmax over the 32 per-core batches, reading scores from
            # PSUM. No max subtraction: |scores| <= sum|v_h| < ~7, so exp is
            # comfortably within fp32 range and softmax is shift-invariant.
            exp_sb = singles.tile([BPC, N], F32)
            ssum = singles.tile([BPC, 1], F32)
            nc.scalar.activation(out=exp_sb, in_=sc_ps, func=AF.Exp,
                                 bias=0.0, scale=1.0, accum_out=ssum[:])
            rsum = singles.tile([BPC, 1], F32)
            nc.vector.reciprocal(out=rsum, in_=ssum)
            out_sb = singles.tile([BPC, N], F32)
            nc.vector.tensor_scalar_mul(out=out_sb, in0=exp_sb, scalar1=rsum[:])
            nc.sync.dma_start(out=out_d[:], in_=out_sb)

    if finalize:
        nc.finalize()
    return nc


def round_fp32r(x):
    """Round fp32 to fp32r (e8m11): round-to-nearest-even, drop low 12 bits."""
    u = np.ascontiguousarray(x, np.float32).view(np.uint32)
    drop = 12
    half = np.uint32(1 << (drop - 1))
    lsb = (u >> drop) & np.uint32(1)
    u2 = (u + (half - np.uint32(1)) + lsb) & np.uint32(~((1 << drop) - 1) & 0xFFFFFFFF)
    return u2.view(np.float32)


def make_in_maps(static_hidden, dynamic_hidden, decoder_hidden, v, W):
    static_hidden = round_fp32r(np.asarray(static_hidden, np.float32))
    dynamic_hidden = round_fp32r(np.asarray(dynamic_hidden, np.float32))
    decoder_hidden = np.asarray(decoder_hidden, np.float32)
    v = np.asarray(v, np.float32)
    W = np.asarray(W, np.float32)

    w0 = W[0]  # (H, 3H)
    wst = round_fp32r(np.ascontiguousarray(w0[:, 0:H].T))   # lhsT, static part
    wdt = round_fp32r(np.ascontiguousarray(w0[:, H:2 * H].T))
    wct = np.ascontiguousarray(w0[:, 2 * H:3 * H].T)
    dect = np.ascontiguousarray(decoder_hidden.T)     # (H, B)

    # vband[:, b, :] is the stationary operand for batch b's v-reduction:
    # v in column b, zeros elsewhere, so lhsT.T @ Wh lands in PSUM row b.
    vband = np.zeros((H, BPC, BPC), np.float32)
    for bb in range(BPC):
        vband[:, bb, bb] = v[0, 0]
    vband = round_fp32r(vband)

    in_maps = []
    for c in range(N_CORES):
        sl = slice(c * BPC, (c + 1) * BPC)
        in_maps.append({
            "static": static_hidden[sl],
            "dynamic": dynamic_hidden[sl],
            "dect": np.ascontiguousarray(dect[:, sl]),
            "wst": wst,
            "wdt": wdt,
            "wct": wct,
            "vband": vband,
        })
    return in_maps


def assemble_output(results):
    outs = [np.asarray(results[c]["out"]) for c in range(N_CORES)]
    return np.concatenate(outs, axis=0)[:, None, :].astype(np.float32)


def kernel(static_hidden, dynamic_hidden, decoder_hidden, v, W):
    nc = build_bass()
    in_maps = make_in_maps(static_hidden, dynamic_hidden, decoder_hidden, v, W)
    res = run_bass_kernel_spmd(nc, in_maps, core_ids=list(range(N_CORES)))
    return assemble_output(res.results)
